# revision 9
# baseline (speedup 1.0000x reference)
"""Trainium2 Bass kernel: bidirectional GNN message passing (scatter-add) + concat.

Computation (per batch b):
    out[b, :, 0:256]   = M_b @ x[b]        where M_b[i, j] = (# edges i<-j) + (# edges j<-i)
    out[b, :, 256:512] = x[b]

M_b is a symmetric count matrix built on the host from the edge indices (pure
index preprocessing; all x-dependent arithmetic runs on the NeuronCores).
Sharding: data-parallel over the batch dim, 4 batches per core on 8 cores.

On-device the scatter half is computed TRANSPOSED: psum[d, i] = sum_j x[j, d] *
M[j, i], with x (f16) as the stationary PE operand -- reused across all 4
i-groups of a j-strip, so LDWEIGHTS is amortized and every matmul streams a
512-wide moving operand. The A strips are shipped from the host already encoded
as fp8e3 (E3M4) holding min(count,31)/2 -- exact for counts <= 31 -- so the PE
consumes the DMA'd bytes directly with ZERO cast instructions; the psum drain
multiplies by 2. Outputs are written f16 (host upcasts; adds ~3e-4 rel err).
The host transposes the [d, n] scatter half back when assembling the output.
"""

import numpy as np

B, N, D = 32, 2048, 256
NC = 8                  # cores
BPC = B // NC           # batches per core = 4
NB = N // 128           # node blocks (j-strips) per batch = 16
DH = D // 128           # d-halves = 2
IG = N // 512           # i-groups of 512 per row = 4
JCH = 4                 # j-strips per A chunk (1 MB DMAs)

_compiled = None


def _build_bass():
    from contextlib import ExitStack
    import concourse.bass as bass
    import concourse.tile as tile
    from concourse import bacc, mybir

    nc = bacc.Bacc("TRN2", target_bir_lowering=False, debug=False, num_devices=NC)
    x_ap = nc.dram_tensor("x", [BPC * N, D], mybir.dt.float32, kind="ExternalInput").ap()
    # a[b, j, i] = e3m4( min(M_b[j, i], 31) / 2 )
    a_ap = nc.dram_tensor("a", [BPC, N, N], mybir.dt.float8e3, kind="ExternalInput").ap()
    # transposed scatter half: ot[b, dh, dd, i] = (M_b @ x_b)[i, dh*128+dd]
    ot_ap = nc.dram_tensor("ot", [BPC, DH, 128, N], mybir.dt.float16, kind="ExternalOutput").ap()
    # x half, same layout as the input
    ox_ap = nc.dram_tensor("ox", [BPC * N, D], mybir.dt.float16, kind="ExternalOutput").ap()

    with tile.TileContext(nc) as tc:
        with ExitStack() as ctx:
            xfpool = ctx.enter_context(tc.tile_pool(name="xf", bufs=2))
            xhpool = ctx.enter_context(tc.tile_pool(name="xh", bufs=1))
            afpool = ctx.enter_context(tc.tile_pool(name="af", bufs=6))
            pspool = ctx.enter_context(tc.tile_pool(name="ps", bufs=8, space="PSUM"))
            otpool = ctx.enter_context(tc.tile_pool(name="ot", bufs=4))

            xw = NB * D  # per-batch x width per partition (node n = g*128 + p)
            x_h = xhpool.tile([128, BPC * xw], mybir.dt.float16)

            def load_x(b):
                # f32 x for batch b -> SBUF, cast to f16, write the x-half of
                # the output (from the f16 copy), f32 tile recycled.
                xf = xfpool.tile([128, xw], mybir.dt.float32, name="xf", tag="xf")
                for q in range(4):
                    qw = xw // 4
                    n0 = b * N + q * (N // 4)
                    nc.sync.dma_start(
                        xf[:, q * qw : (q + 1) * qw],
                        x_ap[n0 : n0 + N // 4].rearrange("(g p) d -> p g d", p=128),
                    )
                    nc.vector.tensor_copy(
                        x_h[:, b * xw + q * qw : b * xw + (q + 1) * qw],
                        xf[:, q * qw : (q + 1) * qw],
                    )
                nc.gpsimd.dma_start(
                    ox_ap[b * N : (b + 1) * N].rearrange("(g p) d -> p g d", p=128),
                    x_h[:, b * xw : (b + 1) * xw],
                )

            for b in range(BPC):
                pending_x = b + 1 if b + 1 < BPC else None
                ps_tiles = {}
                ot_tiles = {}
                for jc in range(NB // JCH):
                    a8 = afpool.tile([128, JCH * N], mybir.dt.float8e3, name="a8", tag="a8")
                    if b == 0 and jc == 0:
                        # split the very first chunk so strip 0 lands quickly
                        nc.sync.dma_start(
                            a8[:, :N],
                            a_ap[0, 0:128].rearrange("(j p) i -> p j i", p=128),
                        )
                        load_x(0)
                        nc.sync.dma_start(
                            a8[:, N:],
                            a_ap[0, 128 : JCH * 128].rearrange("(j p) i -> p j i", p=128),
                        )
                    else:
                        nc.sync.dma_start(
                            a8[:],
                            a_ap[b, jc * JCH * 128 : (jc + 1) * JCH * 128].rearrange(
                                "(j p) i -> p j i", p=128
                            ),
                        )
                    if pending_x is not None and jc == 1:
                        load_x(pending_x)
                    for jj in range(JCH):
                        j = jc * JCH + jj
                        for dh in range(DH):
                            for ig in range(IG):
                                if j == 0:
                                    ps_tiles[(dh, ig)] = pspool.tile(
                                        [128, 512], mybir.dt.float32,
                                        name="ps", tag="ps",
                                    )
                                nc.tensor.matmul(
                                    ps_tiles[(dh, ig)][:],
                                    x_h[
                                        :,
                                        (b * NB + j) * D
                                        + dh * 128 : (b * NB + j) * D
                                        + dh * 128
                                        + 128,
                                    ],
                                    a8[:, jj * N + ig * 512 : jj * N + (ig + 1) * 512],
                                    start=(j == 0),
                                    stop=(j == NB - 1),
                                )
                                if j == NB - 1:
                                    # drain psum -> SBUF (x2 undoes the A/2
                                    # encoding), frees the bank
                                    if ig == 0:
                                        ot_tiles[dh] = otpool.tile(
                                            [128, N], mybir.dt.float16,
                                            name="ot", tag="ot",
                                        )
                                    nc.vector.tensor_scalar_mul(
                                        ot_tiles[dh][:, ig * 512 : (ig + 1) * 512],
                                        ps_tiles[(dh, ig)][:],
                                        2.0,
                                    )
                                    if ig == IG - 1:
                                        nc.gpsimd.dma_start(
                                            ot_ap[b, dh], ot_tiles[dh][:]
                                        )

    nc.compile()
    return nc


def _host_build_counts(batch_idx, src_idx, dst_idx):
    """Per-batch symmetric count matrices, encoded e3m4(min(count,31)/2)."""
    import ml_dtypes

    c = np.arange(256)
    lut = (np.minimum(c, 31) / 2.0).astype(ml_dtypes.float8_e3m4).view(np.uint8)

    a = np.empty((B, N, N), dtype=np.uint8)
    bi = batch_idx.astype(np.int64)
    order = np.argsort(bi, kind="stable")
    bcounts = np.bincount(bi, minlength=B)
    offs = np.zeros(B + 1, dtype=np.int64)
    np.cumsum(bcounts, out=offs[1:])
    src_s = src_idx[order].astype(np.int64)
    dst_s = dst_idx[order].astype(np.int64)
    for b in range(B):
        s = src_s[offs[b] : offs[b + 1]]
        d = dst_s[offs[b] : offs[b + 1]]
        ids = np.concatenate([d * N + s, s * N + d])
        m = np.bincount(ids, minlength=N * N)
        np.minimum(m, 255, out=m)
        a[b] = lut[m.reshape(N, N)]
    return a.view(ml_dtypes.float8_e3m4)


def _make_in_maps(x, batch_idx, src_idx, dst_idx):
    a_all = _host_build_counts(batch_idx, src_idx, dst_idx)
    in_maps = []
    for c in range(NC):
        xs = np.ascontiguousarray(
            x[c * BPC : (c + 1) * BPC].reshape(BPC * N, D).astype(np.float32)
        )
        in_maps.append({"x": xs, "a": np.ascontiguousarray(a_all[c * BPC : (c + 1) * BPC])})
    return in_maps


def kernel(x, batch_idx, src_idx, dst_idx):
    global _compiled
    from concourse import bass_utils

    assert x.shape == (B, N, D), x.shape
    in_maps = _make_in_maps(x, batch_idx, src_idx, dst_idx)

    if _compiled is None:
        _compiled = _build_bass()
    nc = _compiled

    res = bass_utils.run_bass_kernel_spmd(nc, in_maps, core_ids=list(range(NC)))

    out = np.empty((B, N, 2 * D), dtype=np.float32)
    for c in range(NC):
        r = res.results[c]
        # ot [BPC, DH, 128, N] -> [BPC, N, D]
        ot = r["ot"].reshape(BPC, DH, 128, N).astype(np.float32)
        out[c * BPC : (c + 1) * BPC, :, :D] = ot.transpose(0, 3, 1, 2).reshape(BPC, N, D)
        out[c * BPC : (c + 1) * BPC, :, D:] = r["ox"].reshape(BPC, N, D).astype(np.float32)
    return out
